# revision 44
# baseline (speedup 1.0000x reference)
"""Multi-head attention (B=4, S=2048, D=1024, H=16) on 8 trn2 NeuronCores.

Sharding (load-balanced tensor/data parallel):
  Batches are sorted by valid_len and paired heaviest-with-lightest. Core c
  handles batch pair p = c//4 (two batches, "slots" 0/1) and head-quarter
  g = c%4 (4 heads, 256 of the 1024 embedding dims). W_Q/W_K/W_V are
  column-sharded, W_O row-sharded; each core emits one partial transposed
  output per slot batch and the host sums the four partials per batch.

valid_lens specialization: compiled for per-slot k-block count NKB_j =
max over pairs of ceil(valid_len/128). Fully-masked k-blocks (>= NKB_j)
are skipped everywhere; the reference maps masked scores to 1e-9 so each
masked key contributes exp(~0) = 1.0 weight. Their value-sum enters as a
rank-1 PSUM update from msum = xsum @ Wv (xsum = host-precomputed column
sum of the masked X_v rows) and their count enters the denominator the
same way. Partially masked keys inside valid blocks are handled by the
HOST zeroing the masked columns of X_k: the projected k columns are then
exactly 0, so their scores are 0 and exp(0) = 1 = exp(1e-9) -- no
per-partition mask scale needed in the Exp activation.

Per-core dataflow (head-pair units, PE-array packed, software-pipelined):
  - Host passes X^T; q/k produced transposed [e, s]; v natural [s, e].
  - Head pair (2p, 2p+1) lives at partitions 0:64 / 64:128 of the et=p
    slice. Scores matmuls (K=64 contraction) for the two heads are
    row-packed via tile_position (0,*) / (64,*) and run concurrently in
    the PE array. AV matmuls (M=64) are col-packed into one PSUM bank at
    partitions 0:64 / 64:128. Softmax denominators are M=1 ones-matmuls,
    four of them col-packed at partitions 0/32/64/96 of one bank.
  - One-step software pipeline: the kb-step emits scores(kb)+exp(kb),
    then the PREVIOUS step's AV/den matmuls, then fillers. scores(kb+1)
    therefore sits ahead of AV(kb) in the in-order PE queue and the PE
    never stalls at the queue head waiting for exp(kb) -- the Scalar
    engine's exp stream and the PE run fully overlapped.
  - Denominator reciprocals: one exp(-ln(den+cnt)) chain over the four
    packed rows on the Scalar engine, emitted ahead of the next unit's
    exps; the four reciprocal rows are broadcast across partitions with
    K=1 outer-product matmuls (concurrent 32-row strips of the PE array)
    into a scores-pool PSUM tile, and one scalar_tensor_tensor per j
    normalizes ctx for both heads reading that PSUM directly.
  - Scheduling: projections/output-projections are emitted as "fillers"
    popped between attention steps by PE-time budget so the PE stays
    dense (HAM clock-gate at 2.4 GHz) while ACT computes the exps. A few
    junk matmuls warm the PE during the initial DMA; junk LDWEIGHTS keep
    it warm when fillers run dry.
"""

import math
from collections import deque

import numpy as np
import ml_dtypes

import concourse.bass as bass
import concourse.tile as tile
from concourse import mybir
from concourse.bass_utils import run_bass_kernel_spmd

# The walrus build in this container rejects instructions carrying more than
# one semaphore wait ("Too many sync wait commands"), while Tile's scheduler
# freely attaches several. Post-pass: hoist extra waits onto nop instructions
# injected just before the offender on the same engine queue (engines execute
# their queue in order, so the semantics are identical).
def _split_multi_waits(nc, limit=1):
    fn = nc.m.functions[0]
    for b in fn.blocks:
        new = []
        changed = False
        for inst in b.instructions:
            si = inst.sync_info
            waits = list(si.on_wait) if si is not None else []
            if len(waits) > limit:
                for w in waits[:-limit]:
                    nop = mybir.InstNoOp(
                        name=nc.get_next_instruction_name(), ins=[], outs=[]
                    )
                    nop.engine = inst.engine
                    nop.sync_info = mybir.SyncInfo(on_wait=[w], on_update=[])
                    nc.register_instruction(nop)
                    new.append(nop)
                inst.sync_info = mybir.SyncInfo(
                    on_wait=waits[-limit:], on_update=si.on_update
                )
                changed = True
            new.append(inst)
        if changed:
            b.instructions = new


B, S, D, H = 4, 2048, 1024, 16
DH = D // H            # 64 head dim
HL = H // 4            # 4 heads per core
E = HL * DH            # 256 per-core head width
P = 128
SC = 512               # psum bank width in f32 (max matmul N)
SCE = 1024             # attention q-chunk (ACT overhead amortization)
NCH = S // SC          # 4 projection chunks
NCHE = S // SCE        # 2 attention chunks
NSUB = SCE // SC       # matmul sub-chunks per attention chunk
KB = S // P            # 16 k-blocks
DT = D // P            # 8 contraction tiles
ET = E // P            # 2 e-tiles (= head pairs)
OB = D // P            # 8 output-row blocks

BF16 = mybir.dt.bfloat16
F16 = mybir.dt.float16
F32 = mybir.dt.float32
npbf16 = ml_dtypes.bfloat16

# steady-state PE-ns gap per pair-kb step that fillers should cover
FILL_BUDGET = 1900
# one-step software pipeline of AV/den behind scores/exp (bisect toggle)
PIPELINE = True
# exact projected-k width (vs rounding up to 512-col chunks) (bisect toggle)
EXACT_KW = True
# split projection filler groups into two half-groups: DO NOT ENABLE —
# produces intermittent first-run corruption on hardware (stale reads),
# root cause not fully diagnosed; whole-group fillers are race-free
HALVES = False


def build_nc(nkb):
    """nkb = (NKB_0, NKB_1): per-slot k-block counts (1..16)."""
    nc = bass.Bass()
    if EXACT_KW:
        kw = [nkb[sl] * P for sl in range(2)]   # exact projected-k width
    else:
        kw = [min(S, (nkb[sl] * P + SC - 1) // SC * SC) for sl in range(2)]
    kpieces = [
        [(lo, min(lo + SC, kw[sl])) for lo in range(0, kw[sl], SC)]
        for sl in range(2)
    ]
    cnt = [(KB - nkb[sl]) * P for sl in range(2)]

    x_d = {}
    xs_d = {}
    out_d = {}
    for sl in range(2):
        x_d[sl] = [
            nc.dram_tensor(f"x{nm}{sl}", [D, S], BF16, kind="ExternalInput")
            for nm in ("q", "k", "v")
        ]
        xs_d[sl] = nc.dram_tensor(f"xs{sl}", [P, DT], BF16, kind="ExternalInput")
        out_d[sl] = nc.dram_tensor(f"outT{sl}", [D, S], F16, kind="ExternalOutput")
    wq_d = nc.dram_tensor("wq", [D, E], BF16, kind="ExternalInput")
    wk_d = nc.dram_tensor("wk", [D, E], BF16, kind="ExternalInput")
    wv_d = nc.dram_tensor("wv", [D, E], BF16, kind="ExternalInput")
    wo_d = nc.dram_tensor("wo", [E, D], BF16, kind="ExternalInput")

    # nkb-dependent SBUF pressure (kT/v tiles, in KB per partition);
    # trim the elastic pools when large valid_lens would overflow SBUF
    tight = (nkb[0] + nkb[1]) > 24

    with tile.TileContext(nc) as tc:
        with (
            tc.tile_pool(name="wpool", bufs=1) as wpool,
            tc.tile_pool(name="xpool", bufs=24) as xpool,
            tc.tile_pool(name="qkv", bufs=1) as qkv,
            tc.tile_pool(name="expp", bufs=4 if tight else 6) as expp,
            tc.tile_pool(name="ctxsb", bufs=6 if tight else 14) as ctxsb,
            tc.tile_pool(name="recp", bufs=2) as recp,
            tc.tile_pool(name="outp", bufs=2 if tight else 4) as outp,
            tc.tile_pool(name="ps_sc", bufs=2, space="PSUM") as ps_sc,
            tc.tile_pool(name="ps_ctx", bufs=2, space="PSUM") as ps_ctx,
            tc.tile_pool(name="ps_den", bufs=1, space="PSUM") as ps_den,
            tc.tile_pool(name="ps_pj", bufs=1, space="PSUM") as ps_pj,
        ):
            # ---- static SBUF ----
            wq_sb = wpool.tile([P, DT, E], BF16)
            wk_sb = wpool.tile([P, DT, E], BF16)
            wv_sb = wpool.tile([P, DT, E], BF16)
            wo_sb = wpool.tile([P, ET, D], BF16)
            ones_sb = wpool.tile([P, 1], BF16)
            ones32_sb = wpool.tile([P, DH], F32)
            junk_sb = wpool.tile([P, SC], BF16)
            nc.vector.memset(ones_sb, 1.0)
            nc.vector.memset(ones32_sb, 1.0)
            nc.vector.memset(junk_sb, 0.5)
            # dummy activation at t=0: pulls the ~2.7us exp/ln ACT table
            # load into the initial DMA window instead of the first exp
            warm_act = wpool.tile([1, 1], F32)
            nc.scalar.activation(
                warm_act, ones32_sb[0:1, 0:1],
                mybir.ActivationFunctionType.Exp,
            )

            xs_sb = {}
            cnt_sb = {}
            qT_sb = {}
            kT_sb = {}
            v_sb = {}
            ms_sb = {}
            for sl in range(2):
                xs_sb[sl] = wpool.tile([P, DT], BF16, name=f"xs_sb{sl}")
                cnt_sb[sl] = wpool.tile([P, 1], F32, name=f"cnt_sb{sl}")
                nc.vector.memset(cnt_sb[sl], float(cnt[sl]))
                qT_sb[sl] = qkv.tile([P, ET, S], BF16, name=f"qT{sl}")
                kT_sb[sl] = qkv.tile([P, ET, kw[sl]], BF16, name=f"kT{sl}")
                v_sb[sl] = qkv.tile([P, nkb[sl], E], BF16, name=f"v{sl}")
                # msum column per e-tile: partitions = e, used as the STT
                # per-partition scalar in the normalize
                ms_sb[sl] = wpool.tile([P, ET], F32, name=f"ms_sb{sl}")

            # ---- DMA helpers: x tensors load in two column-halves so early
            # projection groups unblock after half the transfer ----
            def load_x(x_dram, width=S, eng=None):
                eng = eng or nc.sync
                h0 = (width // 2 + P - 1) // P * P
                xt = [xpool.tile([P, S], BF16, tag="xt", name="xt")
                      for _ in range(DT)]
                for lo, hi in ((0, h0), (h0, width)):
                    if hi <= lo:
                        continue
                    for dt in range(DT):
                        eng.dma_start(
                            xt[dt][:, lo:hi],
                            x_dram[dt * P : (dt + 1) * P, lo:hi],
                        )
                return xt

            # ---- PE warmup: junk matmuls during the initial DMA window ----
            def warmup(n):
                for _ in range(n):
                    ps = ps_sc.tile([P, SCE], F32, tag="sc", name="wup")
                    nc.tensor.matmul(
                        ps[:, 0:SC], lhsT=junk_sb[:, 0:P], rhs=junk_sb,
                        start=True, stop=True,
                    )

            # ---- filler machinery ----
            # entries: (cost_ns, fn, gate) — a filler may only be popped
            # once `step` (global pair-kb counter) reaches its gate, so
            # fillers whose inputs trail a normalize chain never block the
            # in-order PE queue.
            fillers = deque()
            step = [0]

            def pop_fill(budget):
                oproj_popped = 0
                while budget > 0 and fillers and fillers[0][2] <= step[0]:
                    cost, fn, gate = fillers.popleft()
                    if gate and oproj_popped:
                        fillers.appendleft((cost, fn, gate))
                        break
                    fn()
                    budget -= cost
                    oproj_popped += bool(gate)
                # junk LDWEIGHTS keep the PE HAM-warm when fillers run dry
                for _ in range(min(3, max(0, int(budget // 400)))):
                    nc.tensor.ldweights(weights=junk_sb[:, 0:P])

            # ---- projection groups (w stationary; out [e, s] slices) ----
            def proj_emit(xt, w_sb, out_sb, et, lo, hi, alt=False):
                # alt=True sources the PSUM tile from the (idle at startup)
                # scores pool instead, so back-to-back inline groups do not
                # serialize on the single ps_pj bank's evacuation round-trip
                w = hi - lo
                if alt:
                    ps = ps_sc.tile([P, SCE], F32, tag="sc", name="pja")
                else:
                    ps = ps_pj.tile([P, SC], F32, tag="pj", name="pj")
                for dt in range(DT):
                    nc.tensor.matmul(
                        ps[:, 0:w],
                        lhsT=w_sb[:, dt, et * P : (et + 1) * P],
                        rhs=xt[dt][:, lo:hi],
                        start=(dt == 0),
                        stop=(dt == DT - 1),
                    )
                nc.vector.tensor_copy(out_sb[:, et, lo:hi], ps[:, 0:w])

            def queue_proj(xt, w_sb, out_sb, et, lo, hi, cost=1100):
                if not HALVES:
                    fillers.append(
                        (2 * cost,
                         lambda: proj_emit(xt, w_sb, out_sb, et, lo, hi), 0)
                    )
                    return
                # two half-groups sharing one ps_pj tile; FIFO pop order
                # keeps them consecutive among ps_pj users
                box = {}
                w = hi - lo

                def first():
                    box["ps"] = ps_pj.tile([P, SC], F32, tag="pj", name="pj")
                    for dt in range(DT // 2):
                        nc.tensor.matmul(
                            box["ps"][:, 0:w],
                            lhsT=w_sb[:, dt, et * P : (et + 1) * P],
                            rhs=xt[dt][:, lo:hi],
                            start=(dt == 0),
                            stop=False,
                        )

                def second():
                    for dt in range(DT // 2, DT):
                        nc.tensor.matmul(
                            box["ps"][:, 0:w],
                            lhsT=w_sb[:, dt, et * P : (et + 1) * P],
                            rhs=xt[dt][:, lo:hi],
                            start=False,
                            stop=(dt == DT - 1),
                        )
                    nc.vector.tensor_copy(out_sb[:, et, lo:hi], box["ps"][:, 0:w])

                fillers.append((cost, first, 0))
                fillers.append((cost, second, 0))

            def vproj_emit(xt, sl, st):
                ps = ps_pj.tile([P, SC], F32, tag="pj", name="pj")
                for dt in range(DT):
                    nc.tensor.matmul(
                        ps[:, 0:E],
                        lhsT=xt[dt][:, st * P : (st + 1) * P],
                        rhs=wv_sb[:, dt, :],
                        start=(dt == 0),
                        stop=(dt == DT - 1),
                    )
                nc.vector.tensor_copy(v_sb[sl][:, st, :], ps[:, 0:E])

            def queue_vproj(xt, sl, st, cost=700):
                if not HALVES:
                    fillers.append(
                        (2 * cost, lambda: vproj_emit(xt, sl, st), 0)
                    )
                    return
                box = {}

                def first():
                    box["ps"] = ps_pj.tile([P, SC], F32, tag="pj", name="pj")
                    for dt in range(DT // 2):
                        nc.tensor.matmul(
                            box["ps"][:, 0:E],
                            lhsT=xt[dt][:, st * P : (st + 1) * P],
                            rhs=wv_sb[:, dt, :],
                            start=(dt == 0),
                            stop=False,
                        )

                def second():
                    for dt in range(DT // 2, DT):
                        nc.tensor.matmul(
                            box["ps"][:, 0:E],
                            lhsT=xt[dt][:, st * P : (st + 1) * P],
                            rhs=wv_sb[:, dt, :],
                            start=False,
                            stop=(dt == DT - 1),
                        )
                    nc.vector.tensor_copy(v_sb[sl][:, st, :], box["ps"][:, 0:E])

                fillers.append((cost, first, 0))
                fillers.append((cost, second, 0))

            def ms_group(sl):
                # msum column [E->(P,et)] = Wv.T @ xsum (masked value sums)
                ps = ps_pj.tile([P, SC], F32, tag="pj", name="pj")
                for et in range(ET):
                    for dt in range(DT):
                        nc.tensor.matmul(
                            ps[:, et : et + 1],
                            lhsT=wv_sb[:, dt, et * P : (et + 1) * P],
                            rhs=xs_sb[sl][:, dt : dt + 1],
                            start=(dt == 0),
                            stop=(dt == DT - 1),
                        )
                nc.vector.tensor_copy(ms_sb[sl], ps[:, 0:ET])

            def oproj_group(sl, ch, ob, j, ctx_t, tail_ps=None):
                pool, tag = (tail_ps, "sc") if tail_ps else (ps_pj, "pj")
                ps = pool.tile(
                    [P, SCE] if tail_ps else [P, SC], F32, tag=tag, name="op"
                )
                for et in range(ET):
                    nc.tensor.matmul(
                        ps[:, 0:SC],
                        lhsT=wo_sb[:, et, ob * P : (ob + 1) * P],
                        rhs=ctx_t[(sl, et, j)],
                        start=(et == 0),
                        stop=(et == ET - 1),
                    )
                ost = outp.tile([P, SC], F16, tag="ost", name="ost")
                nc.vector.tensor_copy(ost, ps[:, 0:SC])
                nc.sync.dma_start(
                    out_d[sl][
                        ob * P : (ob + 1) * P,
                        ch * SCE + j * SC : ch * SCE + (j + 1) * SC,
                    ],
                    ost,
                )

            # deferred oproj-filler registration (set after a slot's units,
            # consumed when the slot's last normalize has been emitted)
            post_hook = [None]
            # the normalize STTs are deferred one step so they reach the
            # DVE queue after the broadcast matmuls have completed and
            # never block the ps_pj evacuations behind them
            stt_hook = [None]

            def fire_stt():
                if stt_hook[0] is not None:
                    fn, stt_hook[0] = stt_hook[0], None
                    fn()

            # one-step software pipeline: the AV/den matmuls (and, on the
            # last kb of a unit, the cu/reciprocal epilogue) of step kb are
            # emitted only after scores(kb+1), so the in-order PE queue
            # never stalls on exp(kb) at the queue head.
            pending = [None]

            def flush_pending():
                if pending[0] is not None:
                    fn, pending[0] = pending[0], None
                    fn()

            den_row = {(0, 0): 0, (DH, 0): 32, (0, 1): 64, (DH, 1): 96}

            # ---- attention pair-unit: heads (2p, 2p+1) of slot sl, chunk ch
            def pair_unit(sl, p, ch, ctx_t):
                et = p
                rows = [(0, 2 * p), (DH, 2 * p + 1)]  # (row offset, head)
                # ctx/den PSUM tiles allocated lazily at the first tail
                # emission so the pool rotation matches instruction order
                # (the previous unit's evacuation is emitted in between)
                hold = {}

                def make_tail(kb, ex, first, last):
                    def tail():
                        if first:
                            hold["ctx"] = [
                                ps_ctx.tile([P, SC], F32, tag="ctx",
                                            name="ctx_ps")
                                for _ in range(NSUB)
                            ]
                            hold["den"] = ps_den.tile(
                                [P, SC], F32, tag="den", name="den_ps"
                            )
                        ctx_ps = hold["ctx"]
                        den_ps = hold["den"]
                        for j in range(NSUB):
                            for ro, h in rows:
                                nc.tensor.matmul(
                                    ctx_ps[j][ro : ro + DH, :],
                                    lhsT=v_sb[sl][:, kb, h * DH : (h + 1) * DH],
                                    rhs=ex[ro][:, j * SC : (j + 1) * SC],
                                    start=first,
                                    stop=last,
                                )
                        for j in range(NSUB):
                            for ro, h in rows:
                                r = den_row[(ro, j)]
                                nc.tensor.matmul(
                                    den_ps[r : r + 1, :],
                                    lhsT=ones_sb,
                                    rhs=ex[ro][:, j * SC : (j + 1) * SC],
                                    start=first,
                                    stop=last,
                                    tile_position=(0, r),
                                )
                        if last:
                            finish_unit()
                    return tail

                def finish_unit():
                    ctx_ps = hold["ctx"]
                    den_ps = hold["den"]
                    # Evacuate PSUM promptly: cu = ctx + msum (bf16, SBUF)
                    # frees the ctx banks so the next unit's AVs never wait
                    # on the reciprocal chain below.
                    cu = {}
                    for j in range(NSUB):
                        cu[j] = ctxsb.tile([P, SC], BF16, tag="cu", name="cu")
                        nc.vector.tensor_scalar_add(
                            cu[j], ctx_ps[j], ms_sb[sl][:, et : et + 1]
                        )
                    # 1/(den+cnt) as exp(-ln(den+cnt)) on the Scalar engine:
                    # rides the ACT boundary hole, keeps the vector queue
                    # free, and the ln reads PSUM directly (frees den bank)
                    rec_ln = recp.tile([P, SC], F32, tag="rec", name="rec_ln")
                    nc.scalar.activation(
                        rec_ln[0:97, :],
                        den_ps[0:97, :],
                        mybir.ActivationFunctionType.Ln,
                        bias=cnt_sb[sl][0:97, 0:1],
                    )
                    rec = recp.tile([P, SC], F32, tag="rec", name="rec")
                    nc.scalar.activation(
                        rec[0:97, :],
                        rec_ln[0:97, :],
                        mybir.ActivationFunctionType.Exp,
                        scale=-1.0,
                    )
                    # a couple of fillers here hide the reciprocal's ACT
                    # latency from the PE-queue broadcast matmuls below
                    pop_fill(1200)
                    # broadcast the four reciprocal rows across partitions
                    # with K=1 outer-product matmuls (ones[1,64].T @ rec
                    # row) -- all four land in disjoint 32-row strips of
                    # the PE array and run concurrently. This replaces a
                    # gpsimd stride-0 DMA broadcast that cost 7-11us in
                    # software-DGE descriptor processing.
                    bc_ps = ps_sc.tile([P, SCE], F32, tag="sc", name="bc_ps")
                    for j in range(NSUB):
                        for ro in (0, DH):
                            r = den_row[(ro, j)]
                            nc.tensor.matmul(
                                bc_ps[ro : ro + DH, j * SC : (j + 1) * SC],
                                lhsT=ones32_sb[r : r + 1, 0:DH],
                                rhs=rec[r : r + 1, :],
                                start=True,
                                stop=True,
                                tile_position=(r, ro),
                            )
                    def stt_closure(sl=sl, et=et, cu=cu, bc_ps=bc_ps):
                        for j in range(NSUB):
                            ct = ctxsb.tile([P, SC], BF16, tag="ct",
                                            name="ct")
                            ctx_t[(sl, et, j)] = ct
                            nc.vector.scalar_tensor_tensor(
                                ct,
                                cu[j],
                                1.0,
                                bc_ps[:, j * SC : (j + 1) * SC],
                                mybir.AluOpType.mult,
                                mybir.AluOpType.mult,
                            )
                        if post_hook[0] is not None:
                            hook, post_hook[0] = post_hook[0], None
                            hook()
                    stt_hook[0] = stt_closure

                for kb in range(nkb[sl]):
                    fire_stt()
                    sc_ps = {}
                    ex = {}
                    # j-outer so the two heads' row-packed matmuls sit
                    # adjacent in the PE queue and run concurrently
                    for ro, h in rows:
                        sc_ps[ro] = ps_sc.tile(
                            [P, SCE], F32, tag="sc", name="sc_ps"
                        )
                    for j in range(NSUB):
                        for ro, h in rows:
                            nc.tensor.matmul(
                                sc_ps[ro][:, j * SC : (j + 1) * SC],
                                lhsT=kT_sb[sl][
                                    ro : ro + DH, et, kb * P : (kb + 1) * P
                                ],
                                rhs=qT_sb[sl][
                                    ro : ro + DH,
                                    et,
                                    ch * SCE + j * SC : ch * SCE + (j + 1) * SC,
                                ],
                                start=True,
                                stop=True,
                            )
                    # flush the previous step's AV/den (and, at unit
                    # boundaries, the cu/reciprocal/broadcast chain) BEFORE
                    # emitting this step's exp: the PE queue still sees
                    # scores first, and the ACT queue gets the reciprocal
                    # ahead of this unit's exps so the deferred normalize
                    # STT never stalls the DVE queue
                    step[0] += 1
                    flush_pending()
                    for ro, h in rows:
                        ext = expp.tile([P, SCE], BF16, tag="ex", name="ex")
                        ex[ro] = ext
                        # masked keys inside valid blocks have k == 0 (the
                        # host zeroed those X_k columns) -> score 0 -> 1.0
                        nc.scalar.activation(
                            ext,
                            sc_ps[ro],
                            mybir.ActivationFunctionType.Exp,
                        )
                    tail = make_tail(kb, ex, kb == 0, kb == nkb[sl] - 1)
                    if PIPELINE:
                        pending[0] = tail
                    else:
                        tail()
                    pop_fill(FILL_BUDGET)

            # =========== emission schedule ===========
            # DMAs first (sync queue is FIFO): weights/x for slot 0, then 1.
            nc.sync.dma_start(wq_sb, wq_d.rearrange("(t p) n -> p t n", p=P))
            for sl in range(2):
                nc.sync.dma_start(xs_sb[sl], xs_d[sl][:, :])
            # wv + xv0 on the gpsimd queue, parallel with xq0/xk0 on sync
            nc.gpsimd.dma_start(wv_sb, wv_d.rearrange("(t p) n -> p t n", p=P))
            xt_v0 = load_x(x_d[0][2], width=nkb[0] * P, eng=nc.gpsimd)
            xt_q0 = load_x(x_d[0][0])
            nc.sync.dma_start(wk_sb, wk_d.rearrange("(t p) n -> p t n", p=P))
            xt_k0 = load_x(x_d[0][1], width=kw[0])
            nc.sync.dma_start(wo_sb, wo_d.rearrange("(t p) n -> p t n", p=P))
            # slot-1 loads in consumption order: the slot-1 k AND q
            # projections both feed the first slot-1 scores (~step 27), so
            # xq1 must not trail the whole xv1 transfer on the sync ring
            xt_k1 = load_x(x_d[1][1], width=kw[1])
            xt_q1 = load_x(x_d[1][0])
            xt_v1 = load_x(x_d[1][2], width=nkb[1] * P)

            warmup(32)

            # inline: only what the first attention unit needs right away,
            # alternating PSUM pools so back-to-back groups don't serialize
            # on one bank's evacuation
            alt = [0]

            def proj_in(xt, w_sb, out_sb, et, lo, hi):
                alt[0] ^= 1
                proj_emit(xt, w_sb, out_sb, et, lo, hi, alt=bool(alt[0]))

            nv_in = min(4, nkb[0])
            for st in range(nv_in):
                vproj_emit(xt_v0, 0, st)
            for sc_i in range(2):
                for et in range(ET):
                    proj_in(xt_q0, wq_sb, qT_sb[0], et,
                            sc_i * SC, (sc_i + 1) * SC)
            nk_in = min(2, len(kpieces[0]))
            for kc in range(nk_in):
                for et in range(ET):
                    proj_in(xt_k0, wk_sb, kT_sb[0], et, *kpieces[0][kc])
            ms_group(0)

            # fillers in just-in-time consumption order: remaining slot-0
            # V blocks and K chunks first (needed by later kbs of the first
            # unit), then slot-1 K/Q-chunk0/V/ms (needed by chunk-0 slot-1
            # units), then the chunk-1 q projections
            for st in range(nv_in, nkb[0]):
                queue_vproj(xt_v0, 0, st)
            for kc in range(nk_in, len(kpieces[0])):
                for et in range(ET):
                    queue_proj(xt_k0, wk_sb, kT_sb[0], et, *kpieces[0][kc])
            for kc in range(len(kpieces[1])):
                for et in range(ET):
                    queue_proj(xt_k1, wk_sb, kT_sb[1], et, *kpieces[1][kc])
            for sc_i in range(2):
                for et in range(ET):
                    queue_proj(xt_q1, wq_sb, qT_sb[1], et,
                               sc_i * SC, (sc_i + 1) * SC)
            for st in range(nkb[1]):
                queue_vproj(xt_v1, 1, st)
            fillers.append((600, lambda: ms_group(1), 0))
            for sc_i in range(2, NCH):
                for et in range(ET):
                    queue_proj(xt_q0, wq_sb, qT_sb[0], et,
                               sc_i * SC, (sc_i + 1) * SC)
                    queue_proj(xt_q1, wq_sb, qT_sb[1], et,
                               sc_i * SC, (sc_i + 1) * SC)

            ctx_t = [{}, {}]  # per chunk
            for ch in range(NCHE):
                # last chunk runs slot 1 (short units) first so its output
                # projection is absorbed as fillers by the long slot-0
                # units; only slot 0's oproj remains as the tail
                slots = (0, 1) if ch < NCHE - 1 else (1, 0)
                for sl in slots:
                    for p in range(ET):
                        pair_unit(sl, p, ch, ctx_t[ch])
                    if ch == NCHE - 1 and sl == 0:
                        break  # tail handled below
                    def add_oproj(sl=sl, ch=ch):
                        gate = step[0] + 5
                        for ob in range(OB):
                            for j in range(NSUB):
                                fillers.append(
                                    (440, lambda sl=sl, ch=ch, ob=ob, j=j:
                                     oproj_group(sl, ch, ob, j, ctx_t[ch]),
                                     gate)
                                )
                    post_hook[0] = add_oproj
            flush_pending()
            fire_stt()
            # drain remaining fillers, then the final output projection
            while fillers:
                fillers.popleft()[1]()  # deps are all emitted by now
            for i, (ob, j) in enumerate(
                [(ob, j) for ob in range(OB) for j in range(NSUB)]
            ):
                oproj_group(0, NCHE - 1, ob, j, ctx_t[NCHE - 1],
                            tail_ps=ps_sc if i % 2 else None)

    _split_multi_waits(nc)
    return nc


def plan_shards(valid_lens):
    """Sort batches by valid_len desc, pair heaviest+lightest.

    Returns (pairs, nkb): pairs[p] = (batch_slot0, batch_slot1); nkb[j] is
    the compile-time k-block count for slot j (max over the two pairs)."""
    order = sorted(range(B), key=lambda b: -int(valid_lens[b]))
    pairs = [(order[0], order[3]), (order[1], order[2])]
    nkb = []
    for j in range(2):
        m = max(int(valid_lens[pairs[p][j]]) for p in range(2))
        nkb.append(min(KB, max(1, math.ceil(m / P))))
    return pairs, tuple(nkb)


def make_in_maps(Q, K, V, valid_lens, Wq, Wk, Wv, Wo):
    pairs, nkb = plan_shards(valid_lens)
    xT = {}
    xs = {}
    for b in range(B):
        kT = np.ascontiguousarray(K[b].T).astype(npbf16)
        # zero the masked key columns: projected k is then exactly 0 so
        # masked scores are 0 and exp(0) = 1 = exp(1e-9) (reference mask)
        kT[:, int(valid_lens[b]):] = npbf16(0.0)
        xT[b] = (
            np.ascontiguousarray(Q[b].T).astype(npbf16),
            kT,
            np.ascontiguousarray(V[b].T).astype(npbf16),
        )
    for sl in range(2):
        for p in range(2):
            b = pairs[p][sl]
            xsum = V[b][nkb[sl] * P :, :].sum(axis=0, dtype=np.float64)
            xs[b] = np.ascontiguousarray(
                xsum.reshape(DT, P).T.astype(npbf16)
            )
    wshard = {}
    for g in range(4):
        cols = slice(g * E, (g + 1) * E)
        wshard[g] = (
            (Wq[:, cols] / 8.0).astype(npbf16),
            Wk[:, cols].astype(npbf16),
            Wv[:, cols].astype(npbf16),
            np.ascontiguousarray(Wo[cols, :]).astype(npbf16),
        )
    in_maps = []
    for c in range(8):
        p, g = c // 4, c % 4
        wq, wk, wv, wo = wshard[g]
        m = {"wq": wq, "wk": wk, "wv": wv, "wo": wo}
        for sl in range(2):
            b = pairs[p][sl]
            m[f"xq{sl}"], m[f"xk{sl}"], m[f"xv{sl}"] = xT[b]
            m[f"xs{sl}"] = xs[b]
        in_maps.append(m)
    return in_maps


_NC_CACHE = {}


def kernel(Q, K, V, valid_lens, Wq, Wk, Wv, Wo):
    Q = np.asarray(Q, dtype=np.float32)
    K = np.asarray(K, dtype=np.float32)
    V = np.asarray(V, dtype=np.float32)
    Wq = np.asarray(Wq, dtype=np.float32)
    Wk = np.asarray(Wk, dtype=np.float32)
    Wv = np.asarray(Wv, dtype=np.float32)
    Wo = np.asarray(Wo, dtype=np.float32)
    valid_lens = np.asarray(valid_lens)

    pairs, nkb = plan_shards(valid_lens)
    in_maps = make_in_maps(Q, K, V, valid_lens, Wq, Wk, Wv, Wo)
    if nkb not in _NC_CACHE:
        _NC_CACHE[nkb] = build_nc(nkb)
    nc = _NC_CACHE[nkb]
    res = run_bass_kernel_spmd(nc, in_maps, core_ids=list(range(8)))
    out = np.empty((B, S, D), np.float32)
    for p in range(2):
        for sl in range(2):
            b = pairs[p][sl]
            acc = res.results[4 * p][f"outT{sl}"].astype(np.float32)
            for g in range(1, 4):
                acc += res.results[4 * p + g][f"outT{sl}"].astype(np.float32)
            out[b] = acc.T
    return out


# revision 45
# speedup vs baseline: 1.0053x; 1.0053x over previous
"""Multi-head attention (B=4, S=2048, D=1024, H=16) on 8 trn2 NeuronCores.

Sharding (load-balanced tensor/data parallel):
  Batches are sorted by valid_len and paired heaviest-with-lightest. Core c
  handles batch pair p = c//4 (two batches, "slots" 0/1) and head-quarter
  g = c%4 (4 heads, 256 of the 1024 embedding dims). W_Q/W_K/W_V are
  column-sharded, W_O row-sharded; each core emits one partial transposed
  output per slot batch and the host sums the four partials per batch.

valid_lens specialization: compiled for per-slot k-block count NKB_j =
max over pairs of ceil(valid_len/128). Fully-masked k-blocks (>= NKB_j)
are skipped everywhere; the reference maps masked scores to 1e-9 so each
masked key contributes exp(~0) = 1.0 weight. Their value-sum enters as a
rank-1 PSUM update from msum = xsum @ Wv (xsum = host-precomputed column
sum of the masked X_v rows) and their count enters the denominator the
same way. Partially masked keys inside valid blocks are handled by the
HOST zeroing the masked columns of X_k: the projected k columns are then
exactly 0, so their scores are 0 and exp(0) = 1 = exp(1e-9) -- no
per-partition mask scale needed in the Exp activation.

Per-core dataflow (head-pair units, PE-array packed, software-pipelined):
  - Host passes X^T; q/k produced transposed [e, s]; v natural [s, e].
  - Head pair (2p, 2p+1) lives at partitions 0:64 / 64:128 of the et=p
    slice. Scores matmuls (K=64 contraction) for the two heads are
    row-packed via tile_position (0,*) / (64,*) and run concurrently in
    the PE array. AV matmuls (M=64) are col-packed into one PSUM bank at
    partitions 0:64 / 64:128. Softmax denominators are M=1 ones-matmuls,
    four of them col-packed at partitions 0/32/64/96 of one bank.
  - One-step software pipeline: the kb-step emits scores(kb)+exp(kb),
    then the PREVIOUS step's AV/den matmuls, then fillers. scores(kb+1)
    therefore sits ahead of AV(kb) in the in-order PE queue and the PE
    never stalls at the queue head waiting for exp(kb) -- the Scalar
    engine's exp stream and the PE run fully overlapped.
  - Denominator reciprocals: one exp(-ln(den+cnt)) chain over the four
    packed rows on the Scalar engine, emitted ahead of the next unit's
    exps; the four reciprocal rows are broadcast across partitions with
    K=1 outer-product matmuls (concurrent 32-row strips of the PE array)
    into a scores-pool PSUM tile, and one scalar_tensor_tensor per j
    normalizes ctx for both heads reading that PSUM directly.
  - Scheduling: projections/output-projections are emitted as "fillers"
    popped between attention steps by PE-time budget so the PE stays
    dense (HAM clock-gate at 2.4 GHz) while ACT computes the exps. A few
    junk matmuls warm the PE during the initial DMA; junk LDWEIGHTS keep
    it warm when fillers run dry.
"""

import math
from collections import deque

import numpy as np
import ml_dtypes

import concourse.bass as bass
import concourse.tile as tile
from concourse import mybir
from concourse.bass_utils import run_bass_kernel_spmd

# The walrus build in this container rejects instructions carrying more than
# one semaphore wait ("Too many sync wait commands"), while Tile's scheduler
# freely attaches several. Post-pass: hoist extra waits onto nop instructions
# injected just before the offender on the same engine queue (engines execute
# their queue in order, so the semantics are identical).
def _split_multi_waits(nc, limit=1):
    fn = nc.m.functions[0]
    for b in fn.blocks:
        new = []
        changed = False
        for inst in b.instructions:
            si = inst.sync_info
            waits = list(si.on_wait) if si is not None else []
            if len(waits) > limit:
                for w in waits[:-limit]:
                    nop = mybir.InstNoOp(
                        name=nc.get_next_instruction_name(), ins=[], outs=[]
                    )
                    nop.engine = inst.engine
                    nop.sync_info = mybir.SyncInfo(on_wait=[w], on_update=[])
                    nc.register_instruction(nop)
                    new.append(nop)
                inst.sync_info = mybir.SyncInfo(
                    on_wait=waits[-limit:], on_update=si.on_update
                )
                changed = True
            new.append(inst)
        if changed:
            b.instructions = new


B, S, D, H = 4, 2048, 1024, 16
DH = D // H            # 64 head dim
HL = H // 4            # 4 heads per core
E = HL * DH            # 256 per-core head width
P = 128
SC = 512               # psum bank width in f32 (max matmul N)
SCE = 1024             # attention q-chunk (ACT overhead amortization)
NCH = S // SC          # 4 projection chunks
NCHE = S // SCE        # 2 attention chunks
NSUB = SCE // SC       # matmul sub-chunks per attention chunk
KB = S // P            # 16 k-blocks
DT = D // P            # 8 contraction tiles
ET = E // P            # 2 e-tiles (= head pairs)
OB = D // P            # 8 output-row blocks

BF16 = mybir.dt.bfloat16
F16 = mybir.dt.float16
F32 = mybir.dt.float32
npbf16 = ml_dtypes.bfloat16

# steady-state PE-ns gap per pair-kb step that fillers should cover
FILL_BUDGET = 1900
# one-step software pipeline of AV/den behind scores/exp (bisect toggle)
PIPELINE = True
# exact projected-k width (vs rounding up to 512-col chunks) (bisect toggle)
EXACT_KW = True
# split projection filler groups into two half-groups: DO NOT ENABLE —
# produces intermittent first-run corruption on hardware (stale reads),
# root cause not fully diagnosed; whole-group fillers are race-free
HALVES = False


def build_nc(nkb):
    """nkb = (NKB_0, NKB_1): per-slot k-block counts (1..16)."""
    nc = bass.Bass()
    if EXACT_KW:
        kw = [nkb[sl] * P for sl in range(2)]   # exact projected-k width
    else:
        kw = [min(S, (nkb[sl] * P + SC - 1) // SC * SC) for sl in range(2)]
    kpieces = [
        [(lo, min(lo + SC, kw[sl])) for lo in range(0, kw[sl], SC)]
        for sl in range(2)
    ]
    cnt = [(KB - nkb[sl]) * P for sl in range(2)]

    x_d = {}
    xs_d = {}
    out_d = {}
    for sl in range(2):
        x_d[sl] = [
            nc.dram_tensor(f"x{nm}{sl}", [D, S], BF16, kind="ExternalInput")
            for nm in ("q", "k", "v")
        ]
        xs_d[sl] = nc.dram_tensor(f"xs{sl}", [P, DT], BF16, kind="ExternalInput")
        out_d[sl] = nc.dram_tensor(f"outT{sl}", [D, S], F16, kind="ExternalOutput")
    wq_d = nc.dram_tensor("wq", [D, E], BF16, kind="ExternalInput")
    wk_d = nc.dram_tensor("wk", [D, E], BF16, kind="ExternalInput")
    wv_d = nc.dram_tensor("wv", [D, E], BF16, kind="ExternalInput")
    wo_d = nc.dram_tensor("wo", [E, D], BF16, kind="ExternalInput")

    # nkb-dependent SBUF pressure (kT/v tiles, in KB per partition);
    # trim the elastic pools when large valid_lens would overflow SBUF
    tight = (nkb[0] + nkb[1]) > 24

    with tile.TileContext(nc) as tc:
        with (
            tc.tile_pool(name="wpool", bufs=1) as wpool,
            tc.tile_pool(name="xpool", bufs=24) as xpool,
            tc.tile_pool(name="qkv", bufs=1) as qkv,
            tc.tile_pool(name="expp", bufs=4 if tight else 6) as expp,
            tc.tile_pool(name="ctxsb", bufs=6 if tight else 14) as ctxsb,
            tc.tile_pool(name="recp", bufs=2) as recp,
            tc.tile_pool(name="outp", bufs=2 if tight else 4) as outp,
            tc.tile_pool(name="ps_sc", bufs=2, space="PSUM") as ps_sc,
            tc.tile_pool(name="ps_ctx", bufs=2, space="PSUM") as ps_ctx,
            tc.tile_pool(name="ps_den", bufs=1, space="PSUM") as ps_den,
            tc.tile_pool(name="ps_pj", bufs=1, space="PSUM") as ps_pj,
        ):
            # ---- static SBUF ----
            wq_sb = wpool.tile([P, DT, E], BF16)
            wk_sb = wpool.tile([P, DT, E], BF16)
            wv_sb = wpool.tile([P, DT, E], BF16)
            wo_sb = wpool.tile([P, ET, D], BF16)
            ones_sb = wpool.tile([P, 1], BF16)
            ones32_sb = wpool.tile([P, DH], F32)
            junk_sb = wpool.tile([P, SC], BF16)
            nc.vector.memset(ones_sb, 1.0)
            nc.vector.memset(ones32_sb, 1.0)
            nc.vector.memset(junk_sb, 0.5)
            # dummy activation at t=0: pulls the ~2.7us exp/ln ACT table
            # load into the initial DMA window instead of the first exp
            warm_act = wpool.tile([1, 1], F32)
            nc.scalar.activation(
                warm_act, ones32_sb[0:1, 0:1],
                mybir.ActivationFunctionType.Exp,
            )

            xs_sb = {}
            cnt_sb = {}
            qT_sb = {}
            kT_sb = {}
            v_sb = {}
            ms_sb = {}
            for sl in range(2):
                xs_sb[sl] = wpool.tile([P, DT], BF16, name=f"xs_sb{sl}")
                cnt_sb[sl] = wpool.tile([P, 1], F32, name=f"cnt_sb{sl}")
                nc.vector.memset(cnt_sb[sl], float(cnt[sl]))
                qT_sb[sl] = qkv.tile([P, ET, S], BF16, name=f"qT{sl}")
                kT_sb[sl] = qkv.tile([P, ET, kw[sl]], BF16, name=f"kT{sl}")
                v_sb[sl] = qkv.tile([P, nkb[sl], E], BF16, name=f"v{sl}")
                # msum column per e-tile: partitions = e, used as the STT
                # per-partition scalar in the normalize
                ms_sb[sl] = wpool.tile([P, ET], F32, name=f"ms_sb{sl}")

            # ---- DMA helpers: x tensors load in two column-halves so early
            # projection groups unblock after half the transfer ----
            def load_x(x_dram, width=S, eng=None):
                eng = eng or nc.sync
                h0 = (width // 2 + P - 1) // P * P
                xt = [xpool.tile([P, S], BF16, tag="xt", name="xt")
                      for _ in range(DT)]
                for lo, hi in ((0, h0), (h0, width)):
                    if hi <= lo:
                        continue
                    for dt in range(DT):
                        eng.dma_start(
                            xt[dt][:, lo:hi],
                            x_dram[dt * P : (dt + 1) * P, lo:hi],
                        )
                return xt

            # ---- PE warmup: junk matmuls during the initial DMA window ----
            def warmup(n):
                for _ in range(n):
                    ps = ps_sc.tile([P, SCE], F32, tag="sc", name="wup")
                    nc.tensor.matmul(
                        ps[:, 0:SC], lhsT=junk_sb[:, 0:P], rhs=junk_sb,
                        start=True, stop=True,
                    )

            # ---- filler machinery ----
            # entries: (cost_ns, fn, gate) — a filler may only be popped
            # once `step` (global pair-kb counter) reaches its gate, so
            # fillers whose inputs trail a normalize chain never block the
            # in-order PE queue.
            fillers = deque()
            step = [0]

            def pop_fill(budget):
                oproj_popped = 0
                while budget > 0 and fillers and fillers[0][2] <= step[0]:
                    cost, fn, gate = fillers.popleft()
                    if gate and oproj_popped:
                        fillers.appendleft((cost, fn, gate))
                        break
                    fn()
                    budget -= cost
                    oproj_popped += bool(gate)
                # junk LDWEIGHTS keep the PE HAM-warm when fillers run dry
                for _ in range(min(3, max(0, int(budget // 400)))):
                    nc.tensor.ldweights(weights=junk_sb[:, 0:P])

            # ---- projection groups (w stationary; out [e, s] slices) ----
            def proj_emit(xt, w_sb, out_sb, et, lo, hi, alt=False):
                # alt=True sources the PSUM tile from the (idle at startup)
                # scores pool instead, so back-to-back inline groups do not
                # serialize on the single ps_pj bank's evacuation round-trip
                w = hi - lo
                if alt:
                    ps = ps_sc.tile([P, SCE], F32, tag="sc", name="pja")
                else:
                    ps = ps_pj.tile([P, SC], F32, tag="pj", name="pj")
                for dt in range(DT):
                    nc.tensor.matmul(
                        ps[:, 0:w],
                        lhsT=w_sb[:, dt, et * P : (et + 1) * P],
                        rhs=xt[dt][:, lo:hi],
                        start=(dt == 0),
                        stop=(dt == DT - 1),
                    )
                nc.vector.tensor_copy(out_sb[:, et, lo:hi], ps[:, 0:w])

            def queue_proj(xt, w_sb, out_sb, et, lo, hi, cost=1100):
                if not HALVES:
                    fillers.append(
                        (2 * cost,
                         lambda: proj_emit(xt, w_sb, out_sb, et, lo, hi), 0)
                    )
                    return
                # two half-groups sharing one ps_pj tile; FIFO pop order
                # keeps them consecutive among ps_pj users
                box = {}
                w = hi - lo

                def first():
                    box["ps"] = ps_pj.tile([P, SC], F32, tag="pj", name="pj")
                    for dt in range(DT // 2):
                        nc.tensor.matmul(
                            box["ps"][:, 0:w],
                            lhsT=w_sb[:, dt, et * P : (et + 1) * P],
                            rhs=xt[dt][:, lo:hi],
                            start=(dt == 0),
                            stop=False,
                        )

                def second():
                    for dt in range(DT // 2, DT):
                        nc.tensor.matmul(
                            box["ps"][:, 0:w],
                            lhsT=w_sb[:, dt, et * P : (et + 1) * P],
                            rhs=xt[dt][:, lo:hi],
                            start=False,
                            stop=(dt == DT - 1),
                        )
                    nc.vector.tensor_copy(out_sb[:, et, lo:hi], box["ps"][:, 0:w])

                fillers.append((cost, first, 0))
                fillers.append((cost, second, 0))

            def vproj_emit(xt, sl, st):
                ps = ps_pj.tile([P, SC], F32, tag="pj", name="pj")
                for dt in range(DT):
                    nc.tensor.matmul(
                        ps[:, 0:E],
                        lhsT=xt[dt][:, st * P : (st + 1) * P],
                        rhs=wv_sb[:, dt, :],
                        start=(dt == 0),
                        stop=(dt == DT - 1),
                    )
                nc.vector.tensor_copy(v_sb[sl][:, st, :], ps[:, 0:E])

            def queue_vproj(xt, sl, st, cost=700):
                if not HALVES:
                    fillers.append(
                        (2 * cost, lambda: vproj_emit(xt, sl, st), 0)
                    )
                    return
                box = {}

                def first():
                    box["ps"] = ps_pj.tile([P, SC], F32, tag="pj", name="pj")
                    for dt in range(DT // 2):
                        nc.tensor.matmul(
                            box["ps"][:, 0:E],
                            lhsT=xt[dt][:, st * P : (st + 1) * P],
                            rhs=wv_sb[:, dt, :],
                            start=(dt == 0),
                            stop=False,
                        )

                def second():
                    for dt in range(DT // 2, DT):
                        nc.tensor.matmul(
                            box["ps"][:, 0:E],
                            lhsT=xt[dt][:, st * P : (st + 1) * P],
                            rhs=wv_sb[:, dt, :],
                            start=False,
                            stop=(dt == DT - 1),
                        )
                    nc.vector.tensor_copy(v_sb[sl][:, st, :], box["ps"][:, 0:E])

                fillers.append((cost, first, 0))
                fillers.append((cost, second, 0))

            def ms_group(sl):
                # msum column [E->(P,et)] = Wv.T @ xsum (masked value sums)
                ps = ps_pj.tile([P, SC], F32, tag="pj", name="pj")
                for et in range(ET):
                    for dt in range(DT):
                        nc.tensor.matmul(
                            ps[:, et : et + 1],
                            lhsT=wv_sb[:, dt, et * P : (et + 1) * P],
                            rhs=xs_sb[sl][:, dt : dt + 1],
                            start=(dt == 0),
                            stop=(dt == DT - 1),
                        )
                nc.vector.tensor_copy(ms_sb[sl], ps[:, 0:ET])

            def oproj_group(sl, ch, ob, j, ctx_t, tail_ps=None):
                pool, tag = (tail_ps, "sc") if tail_ps else (ps_pj, "pj")
                ps = pool.tile(
                    [P, SCE] if tail_ps else [P, SC], F32, tag=tag, name="op"
                )
                for et in range(ET):
                    nc.tensor.matmul(
                        ps[:, 0:SC],
                        lhsT=wo_sb[:, et, ob * P : (ob + 1) * P],
                        rhs=ctx_t[(sl, et, j)],
                        start=(et == 0),
                        stop=(et == ET - 1),
                    )
                ost = outp.tile([P, SC], F16, tag="ost", name="ost")
                nc.vector.tensor_copy(ost, ps[:, 0:SC])
                nc.sync.dma_start(
                    out_d[sl][
                        ob * P : (ob + 1) * P,
                        ch * SCE + j * SC : ch * SCE + (j + 1) * SC,
                    ],
                    ost,
                )

            # deferred oproj-filler registration (set after a slot's units,
            # consumed when the slot's last normalize has been emitted)
            post_hook = [None]
            # the normalize STTs are deferred one step so they reach the
            # DVE queue after the broadcast matmuls have completed and
            # never block the ps_pj evacuations behind them
            stt_hook = [None]

            def fire_stt():
                if stt_hook[0] is not None:
                    fn, stt_hook[0] = stt_hook[0], None
                    fn()

            # one-step software pipeline: the AV/den matmuls (and, on the
            # last kb of a unit, the cu/reciprocal epilogue) of step kb are
            # emitted only after scores(kb+1), so the in-order PE queue
            # never stalls on exp(kb) at the queue head.
            pending = [None]

            def flush_pending():
                if pending[0] is not None:
                    fn, pending[0] = pending[0], None
                    fn()

            den_row = {(0, 0): 0, (DH, 0): 32, (0, 1): 64, (DH, 1): 96}

            # ---- attention pair-unit: heads (2p, 2p+1) of slot sl, chunk ch
            def pair_unit(sl, p, ch, ctx_t):
                et = p
                rows = [(0, 2 * p), (DH, 2 * p + 1)]  # (row offset, head)
                # ctx/den PSUM tiles allocated lazily at the first tail
                # emission so the pool rotation matches instruction order
                # (the previous unit's evacuation is emitted in between)
                hold = {}

                def make_tail(kb, ex, first, last):
                    def tail():
                        if first:
                            hold["ctx"] = [
                                ps_ctx.tile([P, SC], F32, tag="ctx",
                                            name="ctx_ps")
                                for _ in range(NSUB)
                            ]
                            hold["den"] = ps_den.tile(
                                [P, SC], F32, tag="den", name="den_ps"
                            )
                        ctx_ps = hold["ctx"]
                        den_ps = hold["den"]
                        for j in range(NSUB):
                            for ro, h in rows:
                                nc.tensor.matmul(
                                    ctx_ps[j][ro : ro + DH, :],
                                    lhsT=v_sb[sl][:, kb, h * DH : (h + 1) * DH],
                                    rhs=ex[ro][:, j * SC : (j + 1) * SC],
                                    start=first,
                                    stop=last,
                                )
                        for j in range(NSUB):
                            for ro, h in rows:
                                r = den_row[(ro, j)]
                                nc.tensor.matmul(
                                    den_ps[r : r + 1, :],
                                    lhsT=ones_sb,
                                    rhs=ex[ro][:, j * SC : (j + 1) * SC],
                                    start=first,
                                    stop=last,
                                    tile_position=(0, r),
                                )
                        if last:
                            finish_unit()
                    return tail

                def finish_unit():
                    ctx_ps = hold["ctx"]
                    den_ps = hold["den"]
                    # Evacuate PSUM promptly: cu = ctx + msum (bf16, SBUF)
                    # frees the ctx banks so the next unit's AVs never wait
                    # on the reciprocal chain below.
                    cu = {}
                    for j in range(NSUB):
                        cu[j] = ctxsb.tile([P, SC], BF16, tag="cu", name="cu")
                        nc.vector.tensor_scalar_add(
                            cu[j], ctx_ps[j], ms_sb[sl][:, et : et + 1]
                        )
                    # 1/(den+cnt) as exp(-ln(den+cnt)) on the Scalar engine:
                    # rides the ACT boundary hole, keeps the vector queue
                    # free, and the ln reads PSUM directly (frees den bank)
                    rec_ln = recp.tile([P, SC], F32, tag="rec", name="rec_ln")
                    nc.scalar.activation(
                        rec_ln[0:97, :],
                        den_ps[0:97, :],
                        mybir.ActivationFunctionType.Ln,
                        bias=cnt_sb[sl][0:97, 0:1],
                    )
                    rec = recp.tile([P, SC], F32, tag="rec", name="rec")
                    nc.scalar.activation(
                        rec[0:97, :],
                        rec_ln[0:97, :],
                        mybir.ActivationFunctionType.Exp,
                        scale=-1.0,
                    )
                    # a couple of fillers here hide the reciprocal's ACT
                    # latency from the PE-queue broadcast matmuls below
                    pop_fill(2600)
                    # broadcast the four reciprocal rows across partitions
                    # with K=1 outer-product matmuls (ones[1,64].T @ rec
                    # row) -- all four land in disjoint 32-row strips of
                    # the PE array and run concurrently. This replaces a
                    # gpsimd stride-0 DMA broadcast that cost 7-11us in
                    # software-DGE descriptor processing.
                    bc_ps = ps_sc.tile([P, SCE], F32, tag="sc", name="bc_ps")
                    for j in range(NSUB):
                        for ro in (0, DH):
                            r = den_row[(ro, j)]
                            nc.tensor.matmul(
                                bc_ps[ro : ro + DH, j * SC : (j + 1) * SC],
                                lhsT=ones32_sb[r : r + 1, 0:DH],
                                rhs=rec[r : r + 1, :],
                                start=True,
                                stop=True,
                                tile_position=(r, ro),
                            )
                    def stt_closure(sl=sl, et=et, cu=cu, bc_ps=bc_ps):
                        for j in range(NSUB):
                            ct = ctxsb.tile([P, SC], BF16, tag="ct",
                                            name="ct")
                            ctx_t[(sl, et, j)] = ct
                            nc.vector.scalar_tensor_tensor(
                                ct,
                                cu[j],
                                1.0,
                                bc_ps[:, j * SC : (j + 1) * SC],
                                mybir.AluOpType.mult,
                                mybir.AluOpType.mult,
                            )
                        if post_hook[0] is not None:
                            hook, post_hook[0] = post_hook[0], None
                            hook()
                    stt_hook[0] = stt_closure

                for kb in range(nkb[sl]):
                    fire_stt()
                    sc_ps = {}
                    ex = {}
                    # j-outer so the two heads' row-packed matmuls sit
                    # adjacent in the PE queue and run concurrently
                    for ro, h in rows:
                        sc_ps[ro] = ps_sc.tile(
                            [P, SCE], F32, tag="sc", name="sc_ps"
                        )
                    for j in range(NSUB):
                        for ro, h in rows:
                            nc.tensor.matmul(
                                sc_ps[ro][:, j * SC : (j + 1) * SC],
                                lhsT=kT_sb[sl][
                                    ro : ro + DH, et, kb * P : (kb + 1) * P
                                ],
                                rhs=qT_sb[sl][
                                    ro : ro + DH,
                                    et,
                                    ch * SCE + j * SC : ch * SCE + (j + 1) * SC,
                                ],
                                start=True,
                                stop=True,
                            )
                    # flush the previous step's AV/den (and, at unit
                    # boundaries, the cu/reciprocal/broadcast chain) BEFORE
                    # emitting this step's exp: the PE queue still sees
                    # scores first, and the ACT queue gets the reciprocal
                    # ahead of this unit's exps so the deferred normalize
                    # STT never stalls the DVE queue
                    step[0] += 1
                    flush_pending()
                    for ro, h in rows:
                        ext = expp.tile([P, SCE], BF16, tag="ex", name="ex")
                        ex[ro] = ext
                        # masked keys inside valid blocks have k == 0 (the
                        # host zeroed those X_k columns) -> score 0 -> 1.0
                        nc.scalar.activation(
                            ext,
                            sc_ps[ro],
                            mybir.ActivationFunctionType.Exp,
                        )
                    tail = make_tail(kb, ex, kb == 0, kb == nkb[sl] - 1)
                    if PIPELINE:
                        pending[0] = tail
                    else:
                        tail()
                    pop_fill(FILL_BUDGET)

            # =========== emission schedule ===========
            # DMAs first (sync queue is FIFO): weights/x for slot 0, then 1.
            nc.sync.dma_start(wq_sb, wq_d.rearrange("(t p) n -> p t n", p=P))
            for sl in range(2):
                nc.sync.dma_start(xs_sb[sl], xs_d[sl][:, :])
            # wv + xv0 on the gpsimd queue, parallel with xq0/xk0 on sync
            nc.gpsimd.dma_start(wv_sb, wv_d.rearrange("(t p) n -> p t n", p=P))
            xt_v0 = load_x(x_d[0][2], width=nkb[0] * P, eng=nc.gpsimd)
            xt_q0 = load_x(x_d[0][0])
            nc.sync.dma_start(wk_sb, wk_d.rearrange("(t p) n -> p t n", p=P))
            xt_k0 = load_x(x_d[0][1], width=kw[0])
            nc.sync.dma_start(wo_sb, wo_d.rearrange("(t p) n -> p t n", p=P))
            # slot-1 loads in consumption order: the slot-1 k AND q
            # projections both feed the first slot-1 scores (~step 27), so
            # xq1 must not trail the whole xv1 transfer on the sync ring
            xt_k1 = load_x(x_d[1][1], width=kw[1])
            xt_q1 = load_x(x_d[1][0])
            xt_v1 = load_x(x_d[1][2], width=nkb[1] * P)

            warmup(32)

            # inline: only what the first attention unit needs right away,
            # alternating PSUM pools so back-to-back groups don't serialize
            # on one bank's evacuation
            alt = [0]

            def proj_in(xt, w_sb, out_sb, et, lo, hi):
                alt[0] ^= 1
                proj_emit(xt, w_sb, out_sb, et, lo, hi, alt=bool(alt[0]))

            nv_in = min(4, nkb[0])
            for st in range(nv_in):
                vproj_emit(xt_v0, 0, st)
            for sc_i in range(2):
                for et in range(ET):
                    proj_in(xt_q0, wq_sb, qT_sb[0], et,
                            sc_i * SC, (sc_i + 1) * SC)
            nk_in = min(2, len(kpieces[0]))
            for kc in range(nk_in):
                for et in range(ET):
                    proj_in(xt_k0, wk_sb, kT_sb[0], et, *kpieces[0][kc])
            ms_group(0)

            # fillers in just-in-time consumption order: remaining slot-0
            # V blocks and K chunks first (needed by later kbs of the first
            # unit), then slot-1 K/Q-chunk0/V/ms (needed by chunk-0 slot-1
            # units), then the chunk-1 q projections
            for st in range(nv_in, nkb[0]):
                queue_vproj(xt_v0, 0, st)
            for kc in range(nk_in, len(kpieces[0])):
                for et in range(ET):
                    queue_proj(xt_k0, wk_sb, kT_sb[0], et, *kpieces[0][kc])
            for kc in range(len(kpieces[1])):
                for et in range(ET):
                    queue_proj(xt_k1, wk_sb, kT_sb[1], et, *kpieces[1][kc])
            for sc_i in range(2):
                for et in range(ET):
                    queue_proj(xt_q1, wq_sb, qT_sb[1], et,
                               sc_i * SC, (sc_i + 1) * SC)
            for st in range(nkb[1]):
                queue_vproj(xt_v1, 1, st)
            fillers.append((600, lambda: ms_group(1), 0))
            for sc_i in range(2, NCH):
                for et in range(ET):
                    queue_proj(xt_q0, wq_sb, qT_sb[0], et,
                               sc_i * SC, (sc_i + 1) * SC)
                    queue_proj(xt_q1, wq_sb, qT_sb[1], et,
                               sc_i * SC, (sc_i + 1) * SC)

            ctx_t = [{}, {}]  # per chunk
            for ch in range(NCHE):
                # last chunk runs slot 1 (short units) first so its output
                # projection is absorbed as fillers by the long slot-0
                # units; only slot 0's oproj remains as the tail
                slots = (0, 1) if ch < NCHE - 1 else (1, 0)
                for sl in slots:
                    for p in range(ET):
                        pair_unit(sl, p, ch, ctx_t[ch])
                    if ch == NCHE - 1 and sl == 0:
                        break  # tail handled below
                    def add_oproj(sl=sl, ch=ch):
                        gate = step[0] + 5
                        for ob in range(OB):
                            for j in range(NSUB):
                                fillers.append(
                                    (440, lambda sl=sl, ch=ch, ob=ob, j=j:
                                     oproj_group(sl, ch, ob, j, ctx_t[ch]),
                                     gate)
                                )
                    post_hook[0] = add_oproj
            flush_pending()
            fire_stt()
            # drain remaining fillers, then the final output projection
            while fillers:
                fillers.popleft()[1]()  # deps are all emitted by now
            # j-major: the first OB groups depend only on the j=0 normalize
            for i, (ob, j) in enumerate(
                [(ob, j) for j in range(NSUB) for ob in range(OB)]
            ):
                oproj_group(0, NCHE - 1, ob, j, ctx_t[NCHE - 1],
                            tail_ps=ps_sc if i % 2 else None)

    _split_multi_waits(nc)
    return nc


def plan_shards(valid_lens):
    """Sort batches by valid_len desc, pair heaviest+lightest.

    Returns (pairs, nkb): pairs[p] = (batch_slot0, batch_slot1); nkb[j] is
    the compile-time k-block count for slot j (max over the two pairs)."""
    order = sorted(range(B), key=lambda b: -int(valid_lens[b]))
    pairs = [(order[0], order[3]), (order[1], order[2])]
    nkb = []
    for j in range(2):
        m = max(int(valid_lens[pairs[p][j]]) for p in range(2))
        nkb.append(min(KB, max(1, math.ceil(m / P))))
    return pairs, tuple(nkb)


def make_in_maps(Q, K, V, valid_lens, Wq, Wk, Wv, Wo):
    pairs, nkb = plan_shards(valid_lens)
    xT = {}
    xs = {}
    for b in range(B):
        kT = np.ascontiguousarray(K[b].T).astype(npbf16)
        # zero the masked key columns: projected k is then exactly 0 so
        # masked scores are 0 and exp(0) = 1 = exp(1e-9) (reference mask)
        kT[:, int(valid_lens[b]):] = npbf16(0.0)
        xT[b] = (
            np.ascontiguousarray(Q[b].T).astype(npbf16),
            kT,
            np.ascontiguousarray(V[b].T).astype(npbf16),
        )
    for sl in range(2):
        for p in range(2):
            b = pairs[p][sl]
            xsum = V[b][nkb[sl] * P :, :].sum(axis=0, dtype=np.float64)
            xs[b] = np.ascontiguousarray(
                xsum.reshape(DT, P).T.astype(npbf16)
            )
    wshard = {}
    for g in range(4):
        cols = slice(g * E, (g + 1) * E)
        wshard[g] = (
            (Wq[:, cols] / 8.0).astype(npbf16),
            Wk[:, cols].astype(npbf16),
            Wv[:, cols].astype(npbf16),
            np.ascontiguousarray(Wo[cols, :]).astype(npbf16),
        )
    in_maps = []
    for c in range(8):
        p, g = c // 4, c % 4
        wq, wk, wv, wo = wshard[g]
        m = {"wq": wq, "wk": wk, "wv": wv, "wo": wo}
        for sl in range(2):
            b = pairs[p][sl]
            m[f"xq{sl}"], m[f"xk{sl}"], m[f"xv{sl}"] = xT[b]
            m[f"xs{sl}"] = xs[b]
        in_maps.append(m)
    return in_maps


_NC_CACHE = {}


def kernel(Q, K, V, valid_lens, Wq, Wk, Wv, Wo):
    Q = np.asarray(Q, dtype=np.float32)
    K = np.asarray(K, dtype=np.float32)
    V = np.asarray(V, dtype=np.float32)
    Wq = np.asarray(Wq, dtype=np.float32)
    Wk = np.asarray(Wk, dtype=np.float32)
    Wv = np.asarray(Wv, dtype=np.float32)
    Wo = np.asarray(Wo, dtype=np.float32)
    valid_lens = np.asarray(valid_lens)

    pairs, nkb = plan_shards(valid_lens)
    in_maps = make_in_maps(Q, K, V, valid_lens, Wq, Wk, Wv, Wo)
    if nkb not in _NC_CACHE:
        _NC_CACHE[nkb] = build_nc(nkb)
    nc = _NC_CACHE[nkb]
    res = run_bass_kernel_spmd(nc, in_maps, core_ids=list(range(8)))
    out = np.empty((B, S, D), np.float32)
    for p in range(2):
        for sl in range(2):
            b = pairs[p][sl]
            acc = res.results[4 * p][f"outT{sl}"].astype(np.float32)
            for g in range(1, 4):
                acc += res.results[4 * p + g][f"outT{sl}"].astype(np.float32)
            out[b] = acc.T
    return out
